# revision 5
# baseline (speedup 1.0000x reference)
"""Trainium2 Bass kernel for CrossAttention3D (single-head, 512-dim, 4x64x64).

Math (per batch b, x = q[b] viewed (C, S)):
    qp = Wq x + bq ; kp = Wk x + bk ; vf = x^T Wv^T + bv
    simT = kp^T qp * C^-0.5 ; E = exp(simT) ; den = colsum(E)
    out = Wo (vf^T E) / den + bo

Sharding: 8 cores = 4 batches x 2 query-halves. Each core projects K/V for
its batch's full 4096 tokens, Q for its own 2048-query half, runs a fused
flash-style attention (no max subtraction: |sim| < 1.5), then the output
projection. Softmax normalization is applied AFTER the (linear) output
projection so the 1/den broadcast is off the PE critical path. No
collectives; host scatters/gathers.

On-chip layouts (partition dim first):
    kp  [128, 4, 4096] (o%128, o//128, s)  bf16 - lhsT for sim^T
    qp  [128, 4, 512]  (o%128, o//128, q)  bf16 - rhs for sim^T (scale folded)
    vf  [128, 32, 512] (s%128, s//128, c)  bf16 - lhsT for E^T@V
    sim^T psum [128(k), 512(q)] -> ACT exp -> et bf16 -> AV + ones-colsum MMs
    All PSUM evictions are on ACT so PE slot-reuse waits merge with data waits.
"""

import numpy as np
import ml_dtypes

import concourse.bass as bass
import concourse.bacc as bacc
import concourse.tile as tile
from concourse import mybir
from concourse.bass_utils import run_bass_kernel_spmd

AF = mybir.ActivationFunctionType
F32 = mybir.dt.float32
BF16 = mybir.dt.bfloat16

B, C, H, W = 4, 512, 64, 64
S = H * W            # 4096 tokens
P = 128              # partitions
CC = C // P          # 4 channel chunks
QH = S // 2          # 2048 queries per core
FB = 512             # free-dim block
NQB = QH // FB       # 4 query blocks per core
NSB = S // FB        # 8 token blocks
NKC = S // P         # 32 key chunks
N_CORES = 8


def _build_bass() -> bass.Bass:
    nc = bacc.Bacc("TRN2", target_bir_lowering=False)

    x_d = nc.dram_tensor("x", [C, S], BF16, kind="ExternalInput")
    xq_d = nc.dram_tensor("xq", [C, QH], BF16, kind="ExternalInput")
    wq_d = nc.dram_tensor("wq", [C, C], BF16, kind="ExternalInput")  # Wq.T * scale
    wk_d = nc.dram_tensor("wk", [C, C], BF16, kind="ExternalInput")  # Wk.T
    wv_d = nc.dram_tensor("wv", [C, C], BF16, kind="ExternalInput")  # Wv.T
    wo_d = nc.dram_tensor("wo", [C, C], BF16, kind="ExternalInput")  # Wo.T
    bq_d = nc.dram_tensor("bq", [C], F32, kind="ExternalInput")      # bq * scale
    bk_d = nc.dram_tensor("bk", [C], F32, kind="ExternalInput")
    bv_d = nc.dram_tensor("bv", [C], F32, kind="ExternalInput")
    bo_d = nc.dram_tensor("bo", [C], F32, kind="ExternalInput")
    out_d = nc.dram_tensor("out", [C, QH], F32, kind="ExternalOutput")

    x_r = x_d[:, :].rearrange("(cc p) s -> p cc s", p=P)
    xq_r = xq_d[:, :].rearrange("(cc p) s -> p cc s", p=P)
    out_r = out_d[:, :].rearrange("(oc p) s -> p oc s", p=P)

    def bcast_ap(ap_1d, parts):
        # [n] -> [parts, n] via 0-stride partition dim (DMA-only pattern)
        return bass.AP(
            tensor=ap_1d.tensor, offset=ap_1d.offset,
            ap=[[0, parts]] + [list(d) for d in ap_1d.ap],
        )

    with tile.TileContext(nc) as tc:
        with (
            tc.tile_pool(name="consts", bufs=1) as consts,
            tc.tile_pool(name="kv", bufs=1) as kv,
            tc.tile_pool(name="xs", bufs=3) as xs,
            tc.tile_pool(name="qps", bufs=2) as qps,
            tc.tile_pool(name="ets", bufs=4) as ets,
            tc.tile_pool(name="xus", bufs=6) as xus,
            tc.tile_pool(name="outs", bufs=4) as outs,
            tc.tile_pool(name="dscr", bufs=2, space="DRAM") as dscr,
            tc.tile_pool(name="psmm", bufs=2, space="PSUM") as psmm,
            tc.tile_pool(name="psav", bufs=4, space="PSUM") as psav,
            tc.tile_pool(name="psden", bufs=2, space="PSUM") as psden,
        ):
            # ---- constants ----
            wq_sb = consts.tile([P, CC, C], BF16, tag="wq")
            wk_sb = consts.tile([P, CC, C], BF16, tag="wk")
            wv_sb = consts.tile([P, CC, C], BF16, tag="wv")
            wo_sb = consts.tile([P, CC, C], BF16, tag="wo")
            for w_sb, w_d in ((wq_sb, wq_d), (wk_sb, wk_d), (wv_sb, wv_d), (wo_sb, wo_d)):
                nc.sync.dma_start(out=w_sb, in_=w_d[:, :].rearrange("(cc p) o -> p cc o", p=P))
            bq_sb = consts.tile([P, CC], F32, tag="bq")
            bk_sb = consts.tile([P, CC], F32, tag="bk")
            bo_sb = consts.tile([P, CC], F32, tag="bo")
            for b_sb, b_d in ((bq_sb, bq_d), (bk_sb, bk_d), (bo_sb, bo_d)):
                nc.sync.dma_start(out=b_sb, in_=b_d[:].rearrange("(cc p) -> p cc", p=P))
            bvb_sb = consts.tile([P, C], F32, tag="bvb")
            nc.sync.dma_start(out=bvb_sb, in_=bcast_ap(bv_d[:], P))
            ones_col = consts.tile([P, 1], BF16, tag="ones_col")
            nc.vector.memset(ones_col, 1.0)

            # ---- persistent K / V ----
            kp_sb = kv.tile([P, CC, S], BF16, tag="kp")
            vf_sb = kv.tile([P, NKC, C], BF16, tag="vf")

            # ---- phase 1: K and V projections over the full sequence ----
            # V first: its Ldweights (lhsT = xt slice) absorbs the xt DMA wait
            # on PE, so the K matmuls that follow carry at most one wait.
            for sb in range(NSB):
                xt = xs.tile([P, CC, FB], BF16, tag="xt")
                nc.sync.dma_start(out=xt, in_=x_r[:, :, sb * FB:(sb + 1) * FB])
                for i4 in range(CC):
                    sc = sb * CC + i4
                    psv = psav.tile([P, FB], F32, tag="av")
                    for cc in range(CC):
                        nc.tensor.matmul(
                            psv, lhsT=xt[:, cc, i4 * P:(i4 + 1) * P],
                            rhs=wv_sb[:, cc, :],
                            start=(cc == 0), stop=(cc == CC - 1),
                        )
                    nc.vector.tensor_add(out=vf_sb[:, sc, :], in0=psv[:], in1=bvb_sb[:])
                for oc in range(CC):
                    ps = psmm.tile([P, FB], F32, tag="mm")
                    for cc in range(CC):
                        nc.tensor.matmul(
                            ps, lhsT=wk_sb[:, cc, oc * P:(oc + 1) * P],
                            rhs=xt[:, cc, :],
                            start=(cc == 0), stop=(cc == CC - 1),
                        )
                    nc.scalar.activation(
                        out=kp_sb[:, oc, sb * FB:(sb + 1) * FB], in_=ps[:],
                        func=AF.Identity, bias=bk_sb[:, oc:oc + 1], scale=1.0,
                    )

            # ---- phase 2: per query-block fused attention ----
            for qb in range(NQB):
                qsl = slice(qb * FB, (qb + 1) * FB)

                # Q projection (scale pre-folded into wq/bq)
                xqt = xs.tile([P, CC, FB], BF16, tag="xt")
                nc.sync.dma_start(out=xqt, in_=xq_r[:, :, qsl])
                qp = qps.tile([P, CC, FB], BF16, tag="qp")
                for oc in range(CC):
                    ps = psmm.tile([P, FB], F32, tag="mm")
                    for cc in range(CC):
                        nc.tensor.matmul(
                            ps, lhsT=wq_sb[:, cc, oc * P:(oc + 1) * P],
                            rhs=xqt[:, cc, :],
                            start=(cc == 0), stop=(cc == CC - 1),
                        )
                    nc.scalar.activation(
                        out=qp[:, oc, :], in_=ps[:],
                        func=AF.Identity, bias=bq_sb[:, oc:oc + 1], scale=1.0,
                    )

                den = psden.tile([1, FB], F32, tag="den")
                avt = [psav.tile([P, FB], F32, tag="av", name=f"avt{qb}_{i}") for i in range(CC)]
                for kc in range(NKC):
                    simt = psmm.tile([P, FB], F32, tag="mm")
                    for oc in range(CC):
                        nc.tensor.matmul(
                            simt, lhsT=kp_sb[:, oc, kc * P:(kc + 1) * P],
                            rhs=qp[:, oc, :],
                            start=(oc == 0), stop=(oc == CC - 1),
                        )
                    et = ets.tile([P, FB], BF16, tag="et")
                    nc.scalar.activation(out=et, in_=simt[:], func=AF.Exp)
                    nc.tensor.matmul(
                        den, lhsT=ones_col[:], rhs=et[:],
                        start=(kc == 0), stop=(kc == NKC - 1),
                    )
                    for c4 in range(CC):
                        nc.tensor.matmul(
                            avt[c4], lhsT=vf_sb[:, kc, c4 * P:(c4 + 1) * P],
                            rhs=et[:],
                            start=(kc == 0), stop=(kc == NKC - 1),
                        )

                # 1/den, broadcast to all partitions via a DRAM bounce; runs
                # concurrently with the output-projection matmuls below.
                den_sb = xs.tile([1, FB], F32, tag="den_sb")
                nc.scalar.activation(out=den_sb, in_=den[:], func=AF.Copy)
                rec = xs.tile([1, FB], F32, tag="rec")
                nc.vector.reciprocal(out=rec, in_=den_sb[:])
                rscr = dscr.tile([1, FB], F32, tag="rscr")
                nc.sync.dma_start(out=rscr, in_=rec[:])
                rbc = xs.tile([P, FB], F32, tag="rbc")
                nc.sync.dma_start(out=rbc, in_=bcast_ap(rscr[0, :], P))

                # evict unnormalized attention output (ACT keeps slot waits
                # mergeable on PE), then project
                xu = [None] * CC
                for c4 in range(CC):
                    xu[c4] = xus.tile([P, FB], BF16, tag="xu", name=f"xu{qb}_{c4}")
                    nc.scalar.activation(out=xu[c4], in_=avt[c4][:], func=AF.Copy)

                for oc in range(CC):
                    po = psmm.tile([P, FB], F32, tag="mm")
                    for c4 in range(CC):
                        nc.tensor.matmul(
                            po, lhsT=wo_sb[:, c4, oc * P:(oc + 1) * P],
                            rhs=xu[c4][:],
                            start=(c4 == 0), stop=(c4 == CC - 1),
                        )
                    yo = outs.tile([P, FB], F32, tag="yo")
                    nc.scalar.activation(out=yo, in_=po[:], func=AF.Copy)
                    # out = yo/den + bo  (normalization commutes with Wo)
                    ot = outs.tile([P, FB], F32, tag="ot")
                    nc.vector.tensor_mul(out=ot, in0=yo[:], in1=rbc[:])
                    nc.vector.tensor_scalar_add(out=ot, in0=ot[:], scalar1=bo_sb[:, oc:oc + 1])
                    nc.sync.dma_start(out=out_r[:, oc, qsl], in_=ot[:])

    nc.finalize()
    return nc


_NC_CACHE = {}


def _get_nc() -> bass.Bass:
    if "nc" not in _NC_CACHE:
        _NC_CACHE["nc"] = _build_bass()
    return _NC_CACHE["nc"]


def make_in_maps(q, Wq, bq, Wk, bk, Wv, bv, Wo, bo):
    f = np.float32
    bf = ml_dtypes.bfloat16
    scale = f(C) ** f(-0.5)
    wq = np.ascontiguousarray((np.asarray(Wq, f).T * scale).astype(bf))
    wk = np.ascontiguousarray(np.asarray(Wk, f).T.astype(bf))
    wv = np.ascontiguousarray(np.asarray(Wv, f).T.astype(bf))
    wo = np.ascontiguousarray(np.asarray(Wo, f).T.astype(bf))
    bqs = np.asarray(bq, f) * scale
    bk = np.asarray(bk, f)
    bv = np.asarray(bv, f)
    bo = np.asarray(bo, f)
    in_maps = []
    for core in range(N_CORES):
        b, half = core // 2, core % 2
        x = np.asarray(q[b], f).reshape(C, S).astype(bf)
        xq = np.ascontiguousarray(x[:, half * QH:(half + 1) * QH])
        in_maps.append({
            "x": np.ascontiguousarray(x), "xq": xq,
            "wq": wq, "wk": wk, "wv": wv, "wo": wo,
            "bq": bqs, "bk": bk, "bv": bv, "bo": bo,
        })
    return in_maps


def gather_out(per_core_outs):
    out = np.zeros((B, C, S), np.float32)
    for core in range(N_CORES):
        b, half = core // 2, core % 2
        out[b, :, half * QH:(half + 1) * QH] = per_core_outs[core]
    return out.reshape(B, C, H, W)


def kernel(q, Wq, bq, Wk, bk, Wv, bv, Wo, bo):
    nc = _get_nc()
    in_maps = make_in_maps(q, Wq, bq, Wk, bk, Wv, bv, Wo, bo)
    res = run_bass_kernel_spmd(nc, in_maps, core_ids=list(range(N_CORES)))
    return gather_out([res.results[i]["out"] for i in range(N_CORES)])
